# revision 16
# baseline (speedup 1.0000x reference)
"""MoE layer (top-1 routing, 3 routed experts + always-on shared expert) on
8 Trainium2 NeuronCores.

Strategy
--------
The reference computes every routed expert densely for every token; top-1
routing means only one of the three routed experts actually contributes per
token. We exploit that sparsity:

- Host (tiny: ~0.01% of FLOPs): gate logits / gumbel softmax / top-1
  argmax + score / load-balance loss, all in float64. Tokens are then
  partitioned per expert and distributed round-robin across the 8 cores.
- Device (99.99% of FLOPs, data-parallel over tokens, weights replicated
  and streamed from HBM): each core runs the shared-expert MLP over its
  1024 tokens plus the three routed-expert MLPs over its gathered slice
  (capacity C per expert) of routed tokens, with the top-1 score applied
  on-device. Matmuls run in fp16 (1 cycle/row on the PE, fp32 PSUM
  accumulate); activations/accumulators in fp32.
- Host combine: place shared outputs, scatter-add routed outputs.

Layout per matmul (out = lhsT.T @ rhs, contraction over partitions):
  layer 1:  h[h,tok]  = W1[d,h-tile].T @ xT[d,tok]   (weights stationary)
  layer 2:  y[tok,d]  = h[h,tok-tile].T @ W2[h,d]    (activations stationary)
so activations stay [hidden, token] in SBUF between layers and no transposes
are needed anywhere (x is fed pre-transposed from host).
"""

import numpy as np

B, T, D = 4, 2048, 2048
H = 8192
HR = 4096
NR = 3
N = B * T
NCORES = 8
TS = N // NCORES          # shared-expert tokens per core
P = 128

_prog_cache = {}


def _build_program(C, caps):
    import concourse.bass as bass
    import concourse.mybir as mybir
    from concourse.tile import TileContext

    import bass_rust

    dt = mybir.dt
    DT = dt.float16
    F32 = dt.float32
    Relu = mybir.ActivationFunctionType.Relu
    Add = mybir.AluOpType.add

    KD = D // P              # 16 k-tiles over model dim
    HT = H // P              # 64 h-tiles (shared)
    HTR = HR // P            # 32 h-tiles (routed)
    HC = 4                   # h-tiles per fused chunk
    DG = D // 512            # 4 output d-groups
    TOKT_S = TS // P         # 8 token tiles (shared)
    TOKT_R = C // P          # token tiles (routed)

    nc = bass.Bass()

    xsT = nc.declare_dram_parameter("xsT", [D, TS], DT, isOutput=False)
    xrT = nc.declare_dram_parameter("xrT", [NR, D, C], DT, isOutput=False)
    w1s = nc.declare_dram_parameter("w1s", [D, H], DT, isOutput=False)
    w2s = nc.declare_dram_parameter("w2s", [H, D], DT, isOutput=False)
    w1r = nc.declare_dram_parameter("w1r", [NR, D, HR], DT, isOutput=False)
    w2r = nc.declare_dram_parameter("w2r", [NR, HR, D], DT, isOutput=False)
    b1s = nc.declare_dram_parameter("b1s", [P, HT], F32, isOutput=False)
    b1r = nc.declare_dram_parameter("b1r", [NR, P, HTR], F32, isOutput=False)
    ys = nc.declare_dram_parameter("ys", [TS, D], F32, isOutput=True)
    yr = nc.declare_dram_parameter("yr", [NR, C, D], F32, isOutput=True)

    with TileContext(nc) as tc:
        with tc.tile_pool(name="const", bufs=1) as const:
            b1s_sb = const.tile([P, HT], F32)
            nc.sync.dma_start(out=b1s_sb[:], in_=b1s[:])
            b1r_sb = const.tile([P, NR, HTR], F32)
            nc.sync.dma_start(out=b1r_sb[:], in_=b1r.rearrange("e p t -> p e t"))

            # pool for routed-expert token tiles opened early so expert 0's
            # gather DMAs overlap the shared-expert compute
            _xr_cm = tc.tile_pool(name="xr", bufs=2 * KD)
            xr_pool = _xr_cm.__enter__()
            xr_tiles = {}

            def load_xr(e):
                ts_ = [xr_pool.tile([P, C], DT, tag="xr", name=f"xr{e}_{k}")
                       for k in range(KD)]
                for k in range(KD):
                    nc.sync.dma_start(out=ts_[k][:],
                                      in_=xrT[e, k * P:(k + 1) * P, :])
                xr_tiles[e] = ts_

            # ---------------- shared expert ----------------
            with tc.tile_pool(name="xs", bufs=1) as xs_pool, \
                 tc.tile_pool(name="w1", bufs=2 * KD) as w1_pool, \
                 tc.tile_pool(name="w2", bufs=2 * HC) as w2_pool, \
                 tc.tile_pool(name="hact", bufs=2 * HC) as h_pool, \
                 tc.tile_pool(name="acc", bufs=TOKT_S) as acc_pool, \
                 tc.tile_pool(name="ps1", bufs=4, space="PSUM") as ps1, \
                 tc.tile_pool(name="ps2", bufs=4, space="PSUM") as ps2:

                # token dim split in halves: the first matmul group only
                # needs the t=0 half, halving the cold-start DMA fill
                xs_sb = [[xs_pool.tile([P, TS // 2], DT, tag=f"xs{k}_{t}",
                                       name=f"xs{k}_{t}") for t in range(2)]
                         for k in range(KD)]
                acc = [acc_pool.tile([P, D], F32, tag="acc", name="acc") for _ in range(TOKT_S)]

                NCH = HT // HC        # 16 chunks
                for c in range(NCH):
                    # layer 1: h_act[hi] = relu(W1.T @ xT + b1), h-chunk c
                    w1t = [w1_pool.tile([P, HC * P], DT, tag="w1", name="w1") for _ in range(KD)]
                    for k in range(KD):
                        nc.sync.dma_start(
                            out=w1t[k][:],
                            in_=w1s[k * P:(k + 1) * P,
                                    c * HC * P:(c + 1) * HC * P])
                        if c == 0:
                            # interleave the one-time activation loads with
                            # chunk 0's weights so the first PE matmuls can
                            # start as soon as the k=0 pair lands
                            nc.sync.dma_start(
                                out=xs_sb[k][0][:],
                                in_=xsT[k * P:(k + 1) * P, 0:TS // 2])
                    if c == 0:
                        for k in range(KD):
                            nc.sync.dma_start(
                                out=xs_sb[k][1][:],
                                in_=xsT[k * P:(k + 1) * P, TS // 2:TS])
                    if c == 0:
                        load_xr(0)
                    h_act = [h_pool.tile([P, TS], DT, tag="h", name="h") for _ in range(HC)]
                    for t in range(TS // 512):
                        for hi in range(HC):
                            ps = ps1.tile([P, 512], F32, tag="s1")
                            for k in range(KD):
                                nc.tensor.matmul(
                                    ps[:],
                                    w1t[k][:, hi * P:(hi + 1) * P],
                                    xs_sb[k][t][:],
                                    start=(k == 0), stop=(k == KD - 1))
                            nc.scalar.activation(
                                h_act[hi][:, t * 512:(t + 1) * 512], ps[:], Relu,
                                bias=b1s_sb[:, c * HC + hi:c * HC + hi + 1])
                    # layer 2: acc[tt][:, dg] += h_act.T @ W2
                    w2t = [w2_pool.tile([P, D], DT, tag="w2", name="w2")
                           for _ in range(HC)]
                    for hk in range(HC):
                        row = (c * HC + hk) * P
                        nc.sync.dma_start(out=w2t[hk][:],
                                          in_=w2s[row:row + P, :])
                    for dg in range(DG):
                        for tt in range(TOKT_S):
                            ps = ps2.tile([P, 512], F32, tag="s2")
                            for hk in range(HC):
                                nc.tensor.matmul(
                                    ps[:],
                                    h_act[hk][:, tt * P:(tt + 1) * P],
                                    w2t[hk][:, dg * 512:(dg + 1) * 512],
                                    start=(hk == 0), stop=(hk == HC - 1))
                            dst = acc[tt][:, dg * 512:(dg + 1) * 512]
                            if c == 0:
                                nc.vector.tensor_copy(out=dst, in_=ps[:])
                            else:
                                nc.vector.tensor_tensor(
                                    out=dst, in0=ps[:], in1=dst, op=Add)
                for tt in range(TOKT_S):
                    nc.gpsimd.dma_start(out=ys[tt * P:(tt + 1) * P, :],
                                        in_=acc[tt][:])

            # ---------------- routed experts ----------------
            with tc.tile_pool(name="w1e", bufs=2 * KD) as w1e_pool, \
                 tc.tile_pool(name="w2e", bufs=2 * HC) as w2e_pool, \
                 tc.tile_pool(name="hacte", bufs=2 * HC) as he_pool, \
                 tc.tile_pool(name="acce", bufs=2 * TOKT_R) as acce_pool, \
                 tc.tile_pool(name="pr1", bufs=4, space="PSUM") as pr1, \
                 tc.tile_pool(name="pr2", bufs=4, space="PSUM") as pr2:

                for e in range(NR):
                    # exact token count for this expert (max over cores);
                    # columns Ce..C of h_act stay garbage and only feed pad
                    # rows of yr that the host never reads
                    Ce = caps[e]
                    if e + 1 < NR:
                        load_xr(e + 1)      # prefetch next expert's tokens
                    xr_sb = xr_tiles.pop(e)
                    acc_r = [acce_pool.tile([P, D], F32, tag="accr", name="accr")
                             for _ in range(TOKT_R)]

                    NCH = HTR // HC       # 8 chunks
                    for c in range(NCH):
                        w1t = [w1e_pool.tile([P, HC * P], DT, tag="w1e",
                                             name="w1e")
                               for _ in range(KD)]
                        for k in range(KD):
                            nc.sync.dma_start(
                                out=w1t[k][:],
                                in_=w1r[e, k * P:(k + 1) * P,
                                        c * HC * P:(c + 1) * HC * P])
                        h_act = [he_pool.tile([P, C], DT, tag="he", name="he")
                                 for _ in range(HC)]
                        for hi in range(HC):
                            ps = pr1.tile([P, C], F32, tag="r1")
                            for k in range(KD):
                                nc.tensor.matmul(
                                    ps[:, :Ce],
                                    w1t[k][:, hi * P:(hi + 1) * P],
                                    xr_sb[k][:, :Ce],
                                    start=(k == 0), stop=(k == KD - 1))
                            hidx = c * HC + hi
                            nc.scalar.activation(
                                h_act[hi][:, :Ce], ps[:, :Ce], Relu,
                                bias=b1r_sb[:, e, hidx:hidx + 1])
                        w2t = [w2e_pool.tile([P, D], DT, tag="w2e", name="w2e")
                               for _ in range(HC)]
                        for hk in range(HC):
                            row = (c * HC + hk) * P
                            nc.sync.dma_start(out=w2t[hk][:],
                                              in_=w2r[e, row:row + P, :])
                        for dg in range(DG):
                            for tt in range(TOKT_R):
                                ps = pr2.tile([P, 512], F32, tag="r2")
                                for hk in range(HC):
                                    nc.tensor.matmul(
                                        ps[:],
                                        h_act[hk][:, tt * P:(tt + 1) * P],
                                        w2t[hk][:, dg * 512:(dg + 1) * 512],
                                        start=(hk == 0), stop=(hk == HC - 1))
                                dst = acc_r[tt][:, dg * 512:(dg + 1) * 512]
                                if c == 0:
                                    nc.vector.tensor_copy(out=dst, in_=ps[:])
                                else:
                                    nc.vector.tensor_tensor(
                                        out=dst, in0=ps[:], in1=dst, op=Add)
                    for tt in range(TOKT_R):
                        nc.gpsimd.dma_start(out=yr[e, tt * P:(tt + 1) * P, :],
                                            in_=acc_r[tt][:])

            _xr_cm.__exit__(None, None, None)

    # walrus accepts at most one semaphore wait per instruction; split the
    # excess Tile-scheduler waits onto standalone same-engine NoOps.
    n_split = 0
    counter = [0]
    for f in nc.m.functions:
        for blk in f.blocks:
            insts = blk.instructions
            new_list = []
            changed = False
            for inst in insts:
                si = inst.sync_info
                if si is not None and len(si.on_wait) > 1:
                    for w in si.on_wait[:-1]:
                        nop = mybir.InstNoOp(
                            name=f"I-waitsplit-{counter[0]}", ins=[], outs=[])
                        counter[0] += 1
                        nop.engine = inst.engine
                        nop.sync_info = bass_rust.SyncInfo(
                            on_wait=[w], on_update=[])
                        new_list.append(nop)
                        n_split += 1
                    inst.sync_info = bass_rust.SyncInfo(
                        on_wait=[si.on_wait[-1]], on_update=si.on_update)
                    changed = True
                new_list.append(inst)
            if changed:
                blk.instructions = new_list
    return nc


def _get_program(C, caps):
    key = (C, tuple(caps))
    if key not in _prog_cache:
        _prog_cache[key] = _build_program(C, caps)
    return _prog_cache[key]


def _gate(x2, noise, gate_w):
    """float64 gate: top-1 index/score per token + load-balance loss."""
    logits = x2.astype(np.float64) @ gate_w.astype(np.float64).T      # [N,NR]
    g = -np.log(-np.log(noise.reshape(N, NR).astype(np.float64) + 1e-9) + 1e-9)
    z = logits + g
    z -= z.max(axis=1, keepdims=True)
    ez = np.exp(z)
    scores = ez / ez.sum(axis=1, keepdims=True)
    idx = np.argmax(scores, axis=1)
    top = scores[np.arange(N), idx].astype(np.float32)
    me = scores.mean(axis=0)
    ce = (scores ** 2).mean(axis=0)
    loss = np.float32((me * ce).sum() * (NR ** 2))
    return idx, top, loss


def kernel(x, noise, gate_w, w1_s, b1_s, w2_s, b2_s, w1_r, b1_r, w2_r, b2_r,
           _trace=False):
    from concourse.bass_utils import run_bass_kernel_spmd

    x2 = np.asarray(x, np.float32).reshape(N, D)
    idx, top, loss = _gate(x2, np.asarray(noise, np.float32),
                           np.asarray(gate_w, np.float32))

    ids = [np.nonzero(idx == e)[0] for e in range(NR)]
    chunks = [[ids[e][c::NCORES] for c in range(NCORES)] for e in range(NR)]
    caps = [max(64, max(len(chunks[e][c]) for c in range(NCORES)))
            for e in range(NR)]
    C = max(P, int(np.ceil(max(caps) / P)) * P)

    nc = _get_program(C, caps)

    f16 = np.float16
    xT16 = x2.T.astype(f16)                       # [D, N]
    common = {
        "w1s": np.ascontiguousarray(np.asarray(w1_s, np.float32).astype(f16)),
        "w2s": np.ascontiguousarray(np.asarray(w2_s, np.float32).astype(f16)),
        "w1r": np.ascontiguousarray(np.asarray(w1_r, np.float32).astype(f16)),
        "w2r": np.ascontiguousarray(np.asarray(w2_r, np.float32).astype(f16)),
        "b1s": np.ascontiguousarray(
            np.asarray(b1_s, np.float32).reshape(H // P, P).T),
        "b1r": np.ascontiguousarray(
            np.asarray(b1_r, np.float32).reshape(NR, HR // P, P)
            .transpose(0, 2, 1)),
    }

    in_maps = []
    for c in range(NCORES):
        xsT_c = np.ascontiguousarray(xT16[:, c * TS:(c + 1) * TS])
        xrT_c = np.zeros((NR, D, C), f16)
        for e in range(NR):
            sel = chunks[e][c]
            xrT_c[e][:, :len(sel)] = xT16[:, sel]
        in_maps.append({"xsT": xsT_c, "xrT": xrT_c, **common})

    res = run_bass_kernel_spmd(nc, in_maps, list(range(NCORES)), trace=_trace)

    out = np.empty((N, D), np.float32)
    for c in range(NCORES):
        out[c * TS:(c + 1) * TS] = res.results[c]["ys"]
    b2s = np.asarray(b2_s, np.float32)
    if b2s.any():
        out += b2s
    b2r = np.asarray(b2_r, np.float32)
    for c in range(NCORES):
        yr_c = res.results[c]["yr"]
        for e in range(NR):
            sel = chunks[e][c]
            add = top[sel, None] * yr_c[e][:len(sel)]
            if b2r[e].any():
                add += top[sel, None] * b2r[e]
            out[sel] += add

    out = out.reshape(B, T, D)
    if _trace:
        kernel.last_result = res
    return out, loss


# revision 17
# speedup vs baseline: 1.1972x; 1.1972x over previous
"""MoE layer (top-1 routing, 3 routed experts + always-on shared expert) on
8 Trainium2 NeuronCores.

Strategy
--------
The reference computes every routed expert densely for every token; top-1
routing means only one of the three routed experts actually contributes per
token. We exploit that sparsity:

- Host (tiny: ~0.01% of FLOPs): gate logits / gumbel softmax / top-1
  argmax + score / load-balance loss, all in float64. Tokens are then
  partitioned per expert and distributed round-robin across the 8 cores.
- Device (99.99% of FLOPs, data-parallel over tokens, weights replicated
  and streamed from HBM): each core runs the shared-expert MLP over its
  1024 tokens plus the three routed-expert MLPs over its gathered slice
  (capacity C per expert) of routed tokens, with the top-1 score applied
  on-device. Matmuls run in fp16 (1 cycle/row on the PE, fp32 PSUM
  accumulate); activations/accumulators in fp32.
- Host combine: place shared outputs, scatter-add routed outputs.

Layout per matmul (out = lhsT.T @ rhs, contraction over partitions):
  layer 1:  h[h,tok]  = W1[d,h-tile].T @ xT[d,tok]   (weights stationary)
  layer 2:  y[tok,d]  = h[h,tok-tile].T @ W2[h,d]    (activations stationary)
so activations stay [hidden, token] in SBUF between layers and no transposes
are needed anywhere (x is fed pre-transposed from host).
"""

import numpy as np

B, T, D = 4, 2048, 2048
H = 8192
HR = 4096
NR = 3
N = B * T
NCORES = 8
TS = N // NCORES          # shared-expert tokens per core
P = 128

_prog_cache = {}


def _build_program(C, caps):
    import concourse.bass as bass
    import concourse.mybir as mybir
    from concourse.tile import TileContext

    import bass_rust

    dt = mybir.dt
    DT = dt.float16
    F32 = dt.float32
    Relu = mybir.ActivationFunctionType.Relu
    Add = mybir.AluOpType.add

    KD = D // P              # 16 k-tiles over model dim
    HT = H // P              # 64 h-tiles (shared)
    HTR = HR // P            # 32 h-tiles (routed)
    HC = 4                   # h-tiles per fused chunk
    DG = D // 512            # 4 output d-groups
    TOKT_S = TS // P         # 8 token tiles (shared)
    TOKT_R = C // P          # token tiles (routed)

    nc = bass.Bass()

    xsT = nc.declare_dram_parameter("xsT", [D, TS], DT, isOutput=False)
    xrT = nc.declare_dram_parameter("xrT", [NR, D, C], DT, isOutput=False)
    w1s = nc.declare_dram_parameter("w1s", [D, H], DT, isOutput=False)
    w2s = nc.declare_dram_parameter("w2s", [H, D], DT, isOutput=False)
    w1r = nc.declare_dram_parameter("w1r", [NR, D, HR], DT, isOutput=False)
    w2r = nc.declare_dram_parameter("w2r", [NR, HR, D], DT, isOutput=False)
    b1s = nc.declare_dram_parameter("b1s", [P, HT], F32, isOutput=False)
    b1r = nc.declare_dram_parameter("b1r", [NR, P, HTR], F32, isOutput=False)
    ys = nc.declare_dram_parameter("ys", [TS, D], F32, isOutput=True)
    yr = nc.declare_dram_parameter("yr", [NR, C, D], F32, isOutput=True)

    with TileContext(nc) as tc:
        with tc.tile_pool(name="const", bufs=1) as const:
            b1s_sb = const.tile([P, HT], F32)
            nc.sync.dma_start(out=b1s_sb[:], in_=b1s[:])
            b1r_sb = const.tile([P, NR, HTR], F32)
            nc.sync.dma_start(out=b1r_sb[:], in_=b1r.rearrange("e p t -> p e t"))

            # pool for routed-expert token tiles opened early so expert 0's
            # gather DMAs overlap the shared-expert compute
            _xr_cm = tc.tile_pool(name="xr", bufs=2 * KD)
            xr_pool = _xr_cm.__enter__()
            xr_tiles = {}

            def load_xr(e):
                ts_ = [xr_pool.tile([P, C], DT, tag="xr", name=f"xr{e}_{k}")
                       for k in range(KD)]
                for k in range(KD):
                    nc.sync.dma_start(out=ts_[k][:],
                                      in_=xrT[e, k * P:(k + 1) * P, :])
                xr_tiles[e] = ts_

            # ---------------- shared expert ----------------
            with tc.tile_pool(name="xs", bufs=1) as xs_pool, \
                 tc.tile_pool(name="w1", bufs=2 * KD) as w1_pool, \
                 tc.tile_pool(name="w2", bufs=2 * HC) as w2_pool, \
                 tc.tile_pool(name="hact", bufs=2 * HC) as h_pool, \
                 tc.tile_pool(name="acc", bufs=TOKT_S) as acc_pool, \
                 tc.tile_pool(name="ps1", bufs=4, space="PSUM") as ps1, \
                 tc.tile_pool(name="ps2", bufs=4, space="PSUM") as ps2:

                xs_sb = [xs_pool.tile([P, TS], DT, tag=f"xs{k}", name=f"xs{k}") for k in range(KD)]
                acc = [acc_pool.tile([P, D], F32, tag="acc", name="acc") for _ in range(TOKT_S)]

                NCH = HT // HC        # 16 chunks
                for c in range(NCH):
                    # layer 1: h_act[hi] = relu(W1.T @ xT + b1), h-chunk c
                    w1t = [w1_pool.tile([P, HC * P], DT, tag="w1", name="w1") for _ in range(KD)]
                    for k in range(KD):
                        nc.sync.dma_start(
                            out=w1t[k][:],
                            in_=w1s[k * P:(k + 1) * P,
                                    c * HC * P:(c + 1) * HC * P])
                        if c == 0:
                            # interleave the one-time activation loads with
                            # chunk 0's weights so the first PE matmuls can
                            # start as soon as the k=0 pair lands
                            nc.sync.dma_start(out=xs_sb[k][:],
                                              in_=xsT[k * P:(k + 1) * P, :])
                    if c == 0:
                        load_xr(0)
                    h_act = [h_pool.tile([P, TS], DT, tag="h", name="h") for _ in range(HC)]
                    for t in range(TS // 512):
                        for hi in range(HC):
                            ps = ps1.tile([P, 512], F32, tag="s1")
                            for k in range(KD):
                                nc.tensor.matmul(
                                    ps[:],
                                    w1t[k][:, hi * P:(hi + 1) * P],
                                    xs_sb[k][:, t * 512:(t + 1) * 512],
                                    start=(k == 0), stop=(k == KD - 1))
                            nc.scalar.activation(
                                h_act[hi][:, t * 512:(t + 1) * 512], ps[:], Relu,
                                bias=b1s_sb[:, c * HC + hi:c * HC + hi + 1])
                    # layer 2: acc[tt][:, dg] += h_act.T @ W2
                    w2t = [w2_pool.tile([P, D], DT, tag="w2", name="w2")
                           for _ in range(HC)]
                    for hk in range(HC):
                        row = (c * HC + hk) * P
                        nc.sync.dma_start(out=w2t[hk][:],
                                          in_=w2s[row:row + P, :])
                    for dg in range(DG):
                        for tt in range(TOKT_S):
                            ps = ps2.tile([P, 512], F32, tag="s2")
                            for hk in range(HC):
                                nc.tensor.matmul(
                                    ps[:],
                                    h_act[hk][:, tt * P:(tt + 1) * P],
                                    w2t[hk][:, dg * 512:(dg + 1) * 512],
                                    start=(hk == 0), stop=(hk == HC - 1))
                            dst = acc[tt][:, dg * 512:(dg + 1) * 512]
                            if c == 0:
                                nc.vector.tensor_copy(out=dst, in_=ps[:])
                            else:
                                nc.vector.tensor_tensor(
                                    out=dst, in0=ps[:], in1=dst, op=Add)
                for tt in range(TOKT_S):
                    nc.gpsimd.dma_start(out=ys[tt * P:(tt + 1) * P, :],
                                        in_=acc[tt][:])

            # ---------------- routed experts ----------------
            with tc.tile_pool(name="w1e", bufs=2 * KD) as w1e_pool, \
                 tc.tile_pool(name="w2e", bufs=2 * HC) as w2e_pool, \
                 tc.tile_pool(name="hacte", bufs=2 * HC) as he_pool, \
                 tc.tile_pool(name="acce", bufs=2 * TOKT_R) as acce_pool, \
                 tc.tile_pool(name="pr1", bufs=4, space="PSUM") as pr1, \
                 tc.tile_pool(name="pr2", bufs=4, space="PSUM") as pr2:

                for e in range(NR):
                    # exact token count for this expert (max over cores);
                    # columns Ce..C of h_act stay garbage and only feed pad
                    # rows of yr that the host never reads
                    Ce = caps[e]
                    if e + 1 < NR:
                        load_xr(e + 1)      # prefetch next expert's tokens
                    xr_sb = xr_tiles.pop(e)
                    acc_r = [acce_pool.tile([P, D], F32, tag="accr", name="accr")
                             for _ in range(TOKT_R)]

                    NCH = HTR // HC       # 8 chunks
                    for c in range(NCH):
                        w1t = [w1e_pool.tile([P, HC * P], DT, tag="w1e",
                                             name="w1e")
                               for _ in range(KD)]
                        for k in range(KD):
                            nc.sync.dma_start(
                                out=w1t[k][:],
                                in_=w1r[e, k * P:(k + 1) * P,
                                        c * HC * P:(c + 1) * HC * P])
                        h_act = [he_pool.tile([P, C], DT, tag="he", name="he")
                                 for _ in range(HC)]
                        for hi in range(HC):
                            ps = pr1.tile([P, C], F32, tag="r1")
                            for k in range(KD):
                                nc.tensor.matmul(
                                    ps[:, :Ce],
                                    w1t[k][:, hi * P:(hi + 1) * P],
                                    xr_sb[k][:, :Ce],
                                    start=(k == 0), stop=(k == KD - 1))
                            hidx = c * HC + hi
                            nc.scalar.activation(
                                h_act[hi][:, :Ce], ps[:, :Ce], Relu,
                                bias=b1r_sb[:, e, hidx:hidx + 1])
                        w2t = [w2e_pool.tile([P, D], DT, tag="w2e", name="w2e")
                               for _ in range(HC)]
                        for hk in range(HC):
                            row = (c * HC + hk) * P
                            nc.sync.dma_start(out=w2t[hk][:],
                                              in_=w2r[e, row:row + P, :])
                        for dg in range(DG):
                            for tt in range(TOKT_R):
                                ps = pr2.tile([P, 512], F32, tag="r2")
                                for hk in range(HC):
                                    nc.tensor.matmul(
                                        ps[:],
                                        h_act[hk][:, tt * P:(tt + 1) * P],
                                        w2t[hk][:, dg * 512:(dg + 1) * 512],
                                        start=(hk == 0), stop=(hk == HC - 1))
                                dst = acc_r[tt][:, dg * 512:(dg + 1) * 512]
                                if c == 0:
                                    nc.vector.tensor_copy(out=dst, in_=ps[:])
                                else:
                                    nc.vector.tensor_tensor(
                                        out=dst, in0=ps[:], in1=dst, op=Add)
                    for tt in range(TOKT_R):
                        nc.gpsimd.dma_start(out=yr[e, tt * P:(tt + 1) * P, :],
                                            in_=acc_r[tt][:])

            _xr_cm.__exit__(None, None, None)

    # walrus accepts at most one semaphore wait per instruction; split the
    # excess Tile-scheduler waits onto standalone same-engine NoOps.
    n_split = 0
    counter = [0]
    for f in nc.m.functions:
        for blk in f.blocks:
            insts = blk.instructions
            new_list = []
            changed = False
            for inst in insts:
                si = inst.sync_info
                if si is not None and len(si.on_wait) > 1:
                    for w in si.on_wait[:-1]:
                        nop = mybir.InstNoOp(
                            name=f"I-waitsplit-{counter[0]}", ins=[], outs=[])
                        counter[0] += 1
                        nop.engine = inst.engine
                        nop.sync_info = bass_rust.SyncInfo(
                            on_wait=[w], on_update=[])
                        new_list.append(nop)
                        n_split += 1
                    inst.sync_info = bass_rust.SyncInfo(
                        on_wait=[si.on_wait[-1]], on_update=si.on_update)
                    changed = True
                new_list.append(inst)
            if changed:
                blk.instructions = new_list
    return nc


def _get_program(C, caps):
    key = (C, tuple(caps))
    if key not in _prog_cache:
        _prog_cache[key] = _build_program(C, caps)
    return _prog_cache[key]


def _gate(x2, noise, gate_w):
    """float64 gate: top-1 index/score per token + load-balance loss."""
    logits = x2.astype(np.float64) @ gate_w.astype(np.float64).T      # [N,NR]
    g = -np.log(-np.log(noise.reshape(N, NR).astype(np.float64) + 1e-9) + 1e-9)
    z = logits + g
    z -= z.max(axis=1, keepdims=True)
    ez = np.exp(z)
    scores = ez / ez.sum(axis=1, keepdims=True)
    idx = np.argmax(scores, axis=1)
    top = scores[np.arange(N), idx].astype(np.float32)
    me = scores.mean(axis=0)
    ce = (scores ** 2).mean(axis=0)
    loss = np.float32((me * ce).sum() * (NR ** 2))
    return idx, top, loss


def kernel(x, noise, gate_w, w1_s, b1_s, w2_s, b2_s, w1_r, b1_r, w2_r, b2_r,
           _trace=False):
    from concourse.bass_utils import run_bass_kernel_spmd

    x2 = np.asarray(x, np.float32).reshape(N, D)
    idx, top, loss = _gate(x2, np.asarray(noise, np.float32),
                           np.asarray(gate_w, np.float32))

    ids = [np.nonzero(idx == e)[0] for e in range(NR)]
    chunks = [[ids[e][c::NCORES] for c in range(NCORES)] for e in range(NR)]
    caps = [max(64, max(len(chunks[e][c]) for c in range(NCORES)))
            for e in range(NR)]
    C = max(P, int(np.ceil(max(caps) / P)) * P)

    nc = _get_program(C, caps)

    f16 = np.float16
    xT16 = x2.T.astype(f16)                       # [D, N]
    common = {
        "w1s": np.ascontiguousarray(np.asarray(w1_s, np.float32).astype(f16)),
        "w2s": np.ascontiguousarray(np.asarray(w2_s, np.float32).astype(f16)),
        "w1r": np.ascontiguousarray(np.asarray(w1_r, np.float32).astype(f16)),
        "w2r": np.ascontiguousarray(np.asarray(w2_r, np.float32).astype(f16)),
        "b1s": np.ascontiguousarray(
            np.asarray(b1_s, np.float32).reshape(H // P, P).T),
        "b1r": np.ascontiguousarray(
            np.asarray(b1_r, np.float32).reshape(NR, HR // P, P)
            .transpose(0, 2, 1)),
    }

    in_maps = []
    for c in range(NCORES):
        xsT_c = np.ascontiguousarray(xT16[:, c * TS:(c + 1) * TS])
        xrT_c = np.zeros((NR, D, C), f16)
        for e in range(NR):
            sel = chunks[e][c]
            xrT_c[e][:, :len(sel)] = xT16[:, sel]
        in_maps.append({"xsT": xsT_c, "xrT": xrT_c, **common})

    res = run_bass_kernel_spmd(nc, in_maps, list(range(NCORES)), trace=_trace)

    out = np.empty((N, D), np.float32)
    for c in range(NCORES):
        out[c * TS:(c + 1) * TS] = res.results[c]["ys"]
    b2s = np.asarray(b2_s, np.float32)
    if b2s.any():
        out += b2s
    b2r = np.asarray(b2_r, np.float32)
    for c in range(NCORES):
        yr_c = res.results[c]["yr"]
        for e in range(NR):
            sel = chunks[e][c]
            add = top[sel, None] * yr_c[e][:len(sel)]
            if b2r[e].any():
                add += top[sel, None] * b2r[e]
            out[sel] += add

    out = out.reshape(B, T, D)
    if _trace:
        kernel.last_result = res
    return out, loss
